# revision 19
# baseline (speedup 1.0000x reference)
"""Multi-head attention + residual + LayerNorm Bass kernel for 8 trn2 NeuronCores.

Sharding: 2 heads per core (head-parallel attention), AllToAll to redistribute
context from head-sharded to sequence-sharded, then each core computes the fc
projection + residual + LayerNorm for its sequence shard.  Outputs (attn probs,
final out) are host-assembled from per-core shards.

Returns (out, attn) exactly like the reference.
"""

import math

import numpy as np

import concourse.bass as bass
import concourse.bacc as bacc
import concourse.tile as tile
from concourse import mybir
from concourse.masks import make_identity

# Fixed problem dimensions (full config)
B, S, D = 4, 2048, 1024
H, DK, DV = 16, 64, 64
NC = 8
HPC = H // NC  # heads per core = 2
LN_EPS = 1e-5

F32 = mybir.dt.float32
F32R = mybir.dt.float32r


def _ceil_div(a, b):
    return (a + b - 1) // b


def build_kernel(mode="causal", b=B, s=S, d=D, n_cores=NC, mm_fp32r=True,
                 write_zeros=False):
    """Build the SPMD Bass program (same program on every core).

    mode: "causal" (skip masked work, rely on pre-zeroed outputs),
          "dense" (no mask at all),
          "general" (read mask values from DRAM input)
    """
    assert d % 128 == 0 and s % 512 == 0
    bs = b * s
    shard = bs // n_cores
    assert shard % 128 == 0
    nkt = s // 128          # k-tiles per batch-row of attention
    nq = s // 128           # q-tiles per batch
    qch = s // 512          # 512-wide q chunks per batch
    ndc = d // 128          # contraction chunks for projections
    d2 = n_cores * 128      # total context dim = H*DV (= d here)
    hd = 128                # per-core head dim (2 heads x 64)

    MDT = F32R if mm_fp32r else F32

    def mm(ap):
        return ap

    nc = bacc.Bacc("TRN2", target_bir_lowering=False, num_devices=n_cores)

    # ---- I/O ----
    xT = nc.dram_tensor("xT", [d, bs], MDT, kind="ExternalInput")
    x_res = nc.dram_tensor("x_res", [shard, d], F32, kind="ExternalInput")
    wq = nc.dram_tensor("wq", [d, hd], MDT, kind="ExternalInput")
    wk = nc.dram_tensor("wk", [d, hd], MDT, kind="ExternalInput")
    wv = nc.dram_tensor("wv", [d, hd], MDT, kind="ExternalInput")
    wfc = nc.dram_tensor("wfc", [d2, d], MDT, kind="ExternalInput")
    gamma = nc.dram_tensor("gamma", [d], F32, kind="ExternalInput")
    beta = nc.dram_tensor("beta", [d], F32, kind="ExternalInput")
    if mode == "general":
        mask_in = nc.dram_tensor("mask", [b, s, s], mybir.dt.uint8,
                                 kind="ExternalInput")

    attn_out = nc.dram_tensor("attn_out", [b, HPC, s, s], F32,
                              kind="ExternalOutput")
    out_shard = nc.dram_tensor("out_shard", [shard, d], F32,
                               kind="ExternalOutput")

    with tile.TileContext(nc) as tc:
        from contextlib import ExitStack
        es = ExitStack()
        with es:
            # ---------- persistent pools ----------
            const_pool = es.enter_context(tc.tile_pool(name="const", bufs=1))
            dram_pool = es.enter_context(
                tc.tile_pool(name="dram", bufs=1, space="DRAM"))

            identity = const_pool.tile([128, 128], F32)
            make_identity(nc, identity)

            if mode == "causal" and write_zeros:
                zero_t = const_pool.tile([128, 128], F32)
                nc.vector.memset(zero_t, 0.0)

            if mode == "causal":
                # 4 additive causal mask variants for the diagonal 512-chunk:
                # M[v][p, f] = 0 if p - f + 128*v >= 0 else -1e9
                cmasks = []
                for v in range(4):
                    cm = const_pool.tile([128, 512], F32, name=f"cmask{v}")
                    nc.gpsimd.memset(cm, 0.0)
                    nc.gpsimd.affine_select(
                        out=cm, in_=cm,
                        compare_op=mybir.AluOpType.is_ge,
                        fill=-1e9,
                        base=128 * v,
                        pattern=[[-1, 512]],
                        channel_multiplier=1,
                    )
                    cmasks.append(cm)

            # a2a buffers
            a2a_in = dram_pool.tile([n_cores, 128, shard], MDT)
            a2a_out = dram_pool.tile([n_cores, 128, shard], MDT)

            qkv_es = es.enter_context(ExitStack())
            qkv_pool = qkv_es.enter_context(tc.tile_pool(name="qkv", bufs=1))

            # Q^T, K^T strips: [128 (2 heads x 64 dk), b*s]
            QT = qkv_pool.tile([128, bs], MDT)
            KT = qkv_pool.tile([128, bs], MDT)
            # V in [k, dv] layout: per batch, [128 k, kt, 128 (2 heads x 64)]
            Vform = qkv_pool.tile([128, b, nkt, 128], MDT)

            # ---------- phase 1: projections ----------
            SC = 512  # s-chunk width for projections
            n_sc = bs // SC
            with tc.tile_pool(name="wqkv", bufs=1) as w_pool, \
                 tc.tile_pool(name="xT_pool", bufs=2) as xT_pool, \
                 tc.tile_pool(name="proj_ps", bufs=4, space="PSUM") as proj_ps, \
                 tc.tile_pool(name="vt_pool", bufs=1) as vt_pool, \
                 tc.tile_pool(name="vt_ps", bufs=2, space="PSUM") as vt_ps:
                # per-core weights, reshaped so that contraction chunks are
                # [128, 128] lhsT tiles
                wq_sb = w_pool.tile([128, ndc, hd], MDT)
                wk_sb = w_pool.tile([128, ndc, hd], MDT)
                wv_sb = w_pool.tile([128, ndc, hd], MDT)
                for w_sb, w_dram in ((wq_sb, wq), (wk_sb, wk), (wv_sb, wv)):
                    nc.sync.dma_start(
                        out=w_sb,
                        in_=w_dram.rearrange("(kc p) m -> p kc m", p=128))
                VT = vt_pool.tile([128, bs], F32)
                for sc in range(n_sc):
                    xt_t = xT_pool.tile([128, ndc, SC], MDT, tag="xt")
                    nc.sync.dma_start(
                        out=xt_t,
                        in_=xT.rearrange("(kc p) n -> p kc n", p=128)[
                            :, :, sc * SC:(sc + 1) * SC])
                    for w_sb, dst in ((wq_sb, QT), (wk_sb, KT), (wv_sb, VT)):
                        for ns in range(SC // 512):
                            ps = proj_ps.tile([128, 512], F32, tag="pp")
                            for kc in range(ndc):
                                nc.tensor.matmul(
                                    ps,
                                    lhsT=mm(w_sb[:, kc, :]),
                                    rhs=mm(xt_t[:, kc, ns * 512:(ns + 1) * 512]),
                                    start=(kc == 0), stop=(kc == ndc - 1))
                            nc.any.tensor_copy(
                                out=dst[:, sc * SC + ns * 512:
                                        sc * SC + (ns + 1) * 512],
                                in_=ps)
                # phase 1b: transpose VT -> Vform ([k, dv] layout)
                for bb in range(b):
                    for g in range(nkt // 4):
                        tp = vt_ps.tile([128, 512], F32, tag="vtp")
                        for j in range(4):
                            kt = g * 4 + j
                            nc.tensor.transpose(
                                tp[:, j * 128:(j + 1) * 128],
                                VT[:, bb * s + kt * 128: bb * s + (kt + 1) * 128],
                                identity)
                        nc.any.tensor_copy(
                            out=Vform[:, bb, g * 4:g * 4 + 4, :],
                            in_=tp.rearrange("p (j m) -> p j m", j=4))

            # ---------- phase 2: attention ----------
            with tc.tile_pool(name="strips", bufs=3) as strip_pool, \
                 tc.tile_pool(name="small", bufs=8) as small_pool, \
                 tc.tile_pool(name="ptbuf", bufs=1) as pt_pool, \
                 tc.tile_pool(name="ctxsb", bufs=3) as ctx_pool, \
                 tc.tile_pool(name="s_ps", bufs=3, space="PSUM") as s_ps, \
                 tc.tile_pool(name="t_ps", bufs=2, space="PSUM") as t_ps, \
                 tc.tile_pool(name="c_ps", bufs=1, space="PSUM") as c_ps, \
                 (tc.tile_pool(name="mload", bufs=4)
                  if mode == "general" else _null_ctx()) as mask_pool:

                for bb in range(b):
                    for qc in range(qch):
                        n_kt_qc = (qc + 1) * 4 if mode == "causal" else nkt
                        PT = [pt_pool.tile([128, nkt, 512], MDT,
                                           name=f"PT{h}", tag=f"PT{h}")
                              for h in range(HPC)]
                        for h in range(HPC):
                            for q4 in range(4):
                                qs = qc * 4 + q4        # q-tile in batch
                                q0 = qs * 128
                                k_act = (qs + 1) * 128 if mode == "causal" else s
                                n_ch = _ceil_div(k_act, 512)
                                strip = strip_pool.tile(
                                    [128, k_act], F32, name="strip", tag="strip")
                                sums = small_pool.tile([128, n_ch], F32,
                                                       name="sums", tag="sums")
                                for ci in range(n_ch):
                                    cw = min(512, k_act - ci * 512)
                                    sp = s_ps.tile([128, cw], F32, tag="sp")
                                    nc.tensor.matmul(
                                        sp,
                                        lhsT=mm(QT[h * 64:(h + 1) * 64,
                                                   bb * s + q0:bb * s + q0 + 128]),
                                        rhs=mm(KT[h * 64:(h + 1) * 64,
                                                  bb * s + ci * 512:
                                                  bb * s + ci * 512 + cw]),
                                        start=True, stop=True)
                                    if mode == "causal" and ci == n_ch - 1:
                                        # diagonal chunk: additive causal mask
                                        nc.vector.tensor_add(
                                            out=sp, in0=sp,
                                            in1=cmasks[qs % 4][:, :cw])
                                    elif mode == "general":
                                        mu8 = mask_pool.tile(
                                            [128, cw], mybir.dt.uint8,
                                            name="mu8", tag="mu8")
                                        nc.sync.dma_start(
                                            out=mu8,
                                            in_=mask_in[bb, q0:q0 + 128,
                                                        ci * 512:ci * 512 + cw])
                                        mf = mask_pool.tile([128, cw], F32,
                                                            name="mf", tag="mf")
                                        nc.scalar.activation(
                                            out=mf, in_=mu8,
                                            func=mybir.ActivationFunctionType.Copy,
                                            scale=-1e9)
                                        nc.vector.tensor_add(
                                            out=sp, in0=sp, in1=mf)
                                    nc.scalar.activation(
                                        out=strip[:, ci * 512:ci * 512 + cw],
                                        in_=sp,
                                        func=mybir.ActivationFunctionType.Exp,
                                        accum_out=sums[:, ci:ci + 1])
                                tot = small_pool.tile([128, 1], F32,
                                                      name="tot", tag="tot")
                                if n_ch > 1:
                                    nc.vector.tensor_reduce(
                                        out=tot, in_=sums,
                                        axis=mybir.AxisListType.X,
                                        op=mybir.AluOpType.add)
                                else:
                                    nc.vector.tensor_copy(out=tot, in_=sums)
                                r = small_pool.tile([128, 1], F32,
                                                    name="r", tag="r")
                                nc.vector.reciprocal(out=r, in_=tot)
                                # normalize strip in place
                                nc.vector.tensor_scalar_mul(
                                    out=strip, in0=strip, scalar1=r)
                                # write attn output rows
                                nc.sync.dma_start(
                                    out=attn_out[bb, h, q0:q0 + 128, 0:k_act],
                                    in_=strip)
                                if mode == "causal" and write_zeros \
                                        and k_act < s:
                                    zt = zero_t[:]
                                    zsrc = bass.AP(
                                        tensor=zt.tensor, offset=zt.offset,
                                        ap=[list(zt.ap[0]),
                                            [0, (s - k_act) // 128],
                                            [1, 128]])
                                    nc.sync.dma_start(
                                        out=attn_out[bb, h, q0:q0 + 128,
                                                     k_act:s],
                                        in_=zsrc)
                                # transpose strip tiles into PT buffer
                                for g in range(_ceil_div(k_act, 512)):
                                    gw = min(4, _ceil_div(k_act, 128) - g * 4)
                                    tp = t_ps.tile([128, gw * 128], F32,
                                                   tag="tp")
                                    for j in range(gw):
                                        kt = g * 4 + j
                                        nc.tensor.transpose(
                                            tp[:, j * 128:(j + 1) * 128],
                                            strip[:, kt * 128:(kt + 1) * 128],
                                            identity)
                                    nc.any.tensor_copy(
                                        out=PT[h][:, g * 4:g * 4 + gw,
                                                  q4 * 128:(q4 + 1) * 128],
                                        in_=tp.rearrange(
                                            "p (j m) -> p j m", j=gw))
                        # AV for this q chunk, one [64, 512] psum per head
                        for h in range(HPC):
                            cps = c_ps.tile([64, 512], F32, tag=f"cps{h}")
                            for kt in range(n_kt_qc):
                                off = (max(0, kt * 128 - qc * 512)
                                       if mode == "causal" else 0)
                                nc.tensor.matmul(
                                    cps[:, off:512],
                                    lhsT=mm(Vform[:, bb, kt,
                                                  h * 64:(h + 1) * 64]),
                                    rhs=mm(PT[h][:, kt, off:512]),
                                    start=(kt == 0), stop=(kt == n_kt_qc - 1),
                                )
                            ctx_sb = ctx_pool.tile([64, 512], MDT,
                                                   tag=f"ctx{h}")
                            nc.any.tensor_copy(out=ctx_sb, in_=cps)
                            # scatter into a2a_in: global col = bb*s + qc*512
                            g0 = bb * s + qc * 512
                            off = 0
                            while off < 512:
                                g = g0 + off
                                j = g // shard
                                c0 = g % shard
                                w = min(512 - off, shard - c0)
                                nc.sync.dma_start(
                                    out=a2a_in[j, h * 64:(h + 1) * 64,
                                               c0:c0 + w],
                                    in_=ctx_sb[:, off:off + w])
                                off += w

                # ---------- phase 3: AllToAll ----------
                nc.gpsimd.collective_compute(
                    "AllToAll",
                    mybir.AluOpType.bypass,
                    replica_groups=[list(range(n_cores))],
                    ins=[a2a_in.opt()],
                    outs=[a2a_out.opt()],
                )

            # release QT/KT/Vform space before the fc phase
            qkv_es.close()

            # ---------- phase 4: fc + residual + LayerNorm ----------
            with tc.tile_pool(name="fc", bufs=1) as fc_pool, \
                 tc.tile_pool(name="fcw", bufs=2) as fcw_pool, \
                 tc.tile_pool(name="fc_ps", bufs=4, space="PSUM") as fc_ps, \
                 tc.tile_pool(name="ln", bufs=4) as ln_pool:
                gamma_sb = fc_pool.tile([128, d], F32)
                beta_sb = fc_pool.tile([128, d], F32)
                g_ap = gamma[:]
                b_ap = beta[:]
                nc.sync.dma_start(out=gamma_sb, in_=bass.AP(
                    tensor=g_ap.tensor, offset=g_ap.offset,
                    ap=[[0, 128]] + list(g_ap.ap)))
                nc.sync.dma_start(out=beta_sb, in_=bass.AP(
                    tensor=b_ap.tensor, offset=b_ap.offset,
                    ap=[[0, 128]] + list(b_ap.ap)))
                eps_sb = fc_pool.tile([128, 1], F32)
                nc.vector.memset(eps_sb, LN_EPS)
                wfc_sb = fc_pool.tile([128, d2 // 128, d], MDT)
                nc.sync.dma_start(
                    out=wfc_sb,
                    in_=wfc.rearrange("(j p) n -> p j n", p=128))
                ctxT_sb = fc_pool.tile([128, d2 // 128, shard], MDT)
                nc.sync.dma_start(
                    out=ctxT_sb,
                    in_=a2a_out.rearrange("j p n -> p j n"))

                for st in range(shard // 128):
                    y = ln_pool.tile([128, d], F32, tag="y")
                    xr = ln_pool.tile([128, d], F32, tag="xr")
                    nc.sync.dma_start(
                        out=xr, in_=x_res[st * 128:(st + 1) * 128, :])
                    for n0 in range(0, d, 512):
                        nw = min(512, d - n0)
                        fps = fc_ps.tile([128, nw], F32, tag="fps")
                        for j in range(d2 // 128):
                            nc.tensor.matmul(
                                fps,
                                lhsT=mm(ctxT_sb[:, j,
                                                st * 128:(st + 1) * 128]),
                                rhs=mm(wfc_sb[:, j, n0:n0 + nw]),
                                start=(j == 0), stop=(j == d2 // 128 - 1))
                        # residual add on eviction
                        nc.vector.tensor_add(
                            out=y[:, n0:n0 + nw],
                            in0=fps,
                            in1=xr[:, n0:n0 + nw])
                    # LayerNorm over free dim d
                    n_sub = _ceil_div(d, 512)
                    sub_w = d // n_sub
                    assert sub_w * n_sub == d and sub_w <= 512
                    stats = ln_pool.tile([128, n_sub, 6], F32, tag="stats")
                    for sg in range(n_sub):
                        nc.vector.bn_stats(
                            out=stats[:, sg, :],
                            in_=y[:, sg * sub_w:(sg + 1) * sub_w])
                    mv = ln_pool.tile([128, 2], F32, tag="mv")
                    nc.vector.bn_aggr(out=mv, in_=stats)
                    std = ln_pool.tile([128, 1], F32, tag="std")
                    nc.scalar.activation(
                        out=std, in_=mv[:, 1:2],
                        func=mybir.ActivationFunctionType.Sqrt,
                        bias=eps_sb, scale=1.0)
                    rstd = ln_pool.tile([128, 1], F32, tag="rstd")
                    nc.vector.reciprocal(out=rstd, in_=std)
                    # (y - mean) * rstd
                    nc.vector.tensor_scalar(
                        out=y, in0=y,
                        scalar1=mv[:, 0:1], scalar2=rstd,
                        op0=mybir.AluOpType.subtract,
                        op1=mybir.AluOpType.mult)
                    nc.vector.tensor_mul(out=y, in0=y, in1=gamma_sb)
                    nc.vector.tensor_add(out=y, in0=y, in1=beta_sb)
                    nc.sync.dma_start(
                        out=out_shard[st * 128:(st + 1) * 128, :], in_=y)

    nc.compile()
    return nc


class _null_ctx:
    def __enter__(self):
        return None

    def __exit__(self, *a):
        return False


# ---------------------------------------------------------------------------
# host wrapper
# ---------------------------------------------------------------------------

_cache = {}


def _get_nc(mode):
    if mode not in _cache:
        _cache[mode] = build_kernel(mode=mode)
    return _cache[mode]


def _detect_mode(mask):
    if not mask.any():
        return "dense"
    # causal: mask[b, i, j] == (j > i) for all b
    s = mask.shape[-1]
    causal = np.triu(np.ones((s, s), dtype=bool), k=1)
    if all(np.array_equal(mask[i], causal) for i in range(mask.shape[0])):
        return "causal"
    return "general"


def make_in_maps(x, attention_mask, w_q, w_k, w_v, w_fc, ln_gamma, ln_beta,
                 mode, n_cores=NC):
    b, s, d = x.shape
    bs = b * s
    shard = bs // n_cores
    xf = np.ascontiguousarray(x.reshape(bs, d).astype(np.float32))
    xT = np.ascontiguousarray(xf.T)
    scale = np.float32(1.0 / math.sqrt(DK))
    in_maps = []
    for c in range(n_cores):
        cols = slice(c * HPC * DK, (c + 1) * HPC * DK)
        m = {
            "xT": xT,
            "x_res": np.ascontiguousarray(xf[c * shard:(c + 1) * shard]),
            "wq": np.ascontiguousarray(w_q[:, cols] * scale),
            "wk": np.ascontiguousarray(w_k[:, cols]),
            "wv": np.ascontiguousarray(w_v[:, cols]),
            "wfc": np.ascontiguousarray(w_fc.astype(np.float32)),
            "gamma": np.ascontiguousarray(ln_gamma.astype(np.float32)),
            "beta": np.ascontiguousarray(ln_beta.astype(np.float32)),
        }
        if mode == "general":
            m["mask"] = np.ascontiguousarray(
                attention_mask.astype(np.uint8))
        in_maps.append(m)
    return in_maps


def assemble(results, b=B, s=S, d=D, n_cores=NC):
    h = HPC * n_cores
    attn = np.empty((b, h, s, s), dtype=np.float32)
    outs = []
    shard = b * s // n_cores
    for c in range(n_cores):
        attn[:, c * HPC:(c + 1) * HPC] = \
            np.asarray(results[c]["attn_out"]).reshape(b, HPC, s, s)
        outs.append(np.asarray(results[c]["out_shard"]).reshape(shard, d))
    out = np.concatenate(outs, axis=0).reshape(b, s, d)
    return out, attn


def kernel(x, attention_mask, w_q, w_k, w_v, w_fc, ln_gamma, ln_beta):
    from concourse.bass_utils import run_bass_kernel_spmd
    x = np.asarray(x, dtype=np.float32)
    attention_mask = np.asarray(attention_mask).astype(bool)
    mode = _detect_mode(attention_mask)
    nc = _get_nc(mode)
    in_maps = make_in_maps(x, attention_mask, np.asarray(w_q), np.asarray(w_k),
                           np.asarray(w_v), np.asarray(w_fc),
                           np.asarray(ln_gamma), np.asarray(ln_beta), mode)
    res = run_bass_kernel_spmd(nc, in_maps, list(range(NC)))
    return assemble(res.results)
